# revision 20
# baseline (speedup 1.0000x reference)
"""DitLinearTemporalSelfAttention on 8 TRN2 NeuronCores (Bass/Tile).

Sharding: token-parallel. Core c handles batch b=c//2, token half c%2
(2048 tokens, full D=1024). The temporal-softmax/context reduction over
T=4096 spans two cores per batch -> pairwise AllReduce [[0,1],[2,3],...]
of the tiny per-batch [H,dh,dh+1] context+ksum buffer (266 KB).

Host-side folding: layernorm affine into Wq/Wk/Wv, the emb MLP
(silu(emb)@emb_W+emb_b -> scale/shift -> s2/sh2) in numpy, gate_msa
folded into out_W per batch. Device math per core:
  xn   = LN(x) -> bf16 -> PE transpose -> xnT
  k/v:  out[t,j] accumulated dc-outer into 4 psum banks, exp fused
  ctx_unnorm[h,d,l] = sum_t expk[t,d] * v[t,l]; ksum via ones col of v
  (pairwise AllReduce) -> q proj overlaps collective -> ctx normalize
  y: 4 same-parity heads per psum bank (one accumulation group), strided
     batched reciprocal + one broadcast multiply per bank
  LN(y) on vector (batched sqrt across 4-tile superblocks), styl on
  gpsimd+vector, silu -> bf16 -> transpose -> out matmul with the x
  residual added in PSUM via an fp32r identity matmul
"""

import numpy as np

import concourse.bass as bass
import concourse.bacc as bacc
import concourse.mybir as mybir
import concourse.tile as tile
from concourse import masks
from concourse.bass_utils import run_bass_kernel_spmd

B, T, D, H, DH = 4, 4096, 1024, 16, 64
NCORES = 8
TL = T // 2          # tokens per core
NT = TL // 128       # 16 token tiles
ND = D // 128        # 8 d-chunks
EPS = 1e-5
FP32 = mybir.dt.float32
FP32R = mybir.dt.float32r
BF16 = mybir.dt.bfloat16
AF = mybir.ActivationFunctionType
OP = mybir.AluOpType

_CACHE = {}
USE_COLLECTIVE = True
Y_GROUPED = True   # 4 same-parity y-heads per psum bank (one accum group)


def r32(ap):
    return ap.bitcast(FP32R)


def _legalize_waits(nc, cap=2, escap=2):
    """Split >cap semaphore waits off any instruction into EventSemaphore
    instructions placed immediately before it on the same engine (walrus
    codegen structs hold only a few sync-wait slots)."""
    n = 0
    for bb in nc.main_func.blocks:
        out = []
        changed = False
        for ins in bb.instructions:
            si = ins.sync_info
            ty = type(ins).__name__
            icap = 1 if ty == "InstDMACopy" else cap
            if (si is not None and si.on_wait is not None
                    and len(si.on_wait) > icap
                    and ty not in ("InstDrain", "InstEventSemaphore")):
                waits = list(si.on_wait)
                keep, extra = waits[:icap], waits[icap:]
                while extra:
                    chunk, extra = extra[:escap], extra[escap:]
                    n += 1
                    es = mybir.InstEventSemaphore(
                        name=f"I-wsplit-{n}", engine=ins.engine,
                        sync_info=mybir.SyncInfo(on_wait=list(chunk),
                                                 on_update=[]))
                    out.append(es)
                ins.sync_info = mybir.SyncInfo(
                    on_wait=keep, on_update=list(si.on_update or []))
                changed = True
            out.append(ins)
        if changed:
            bb.instructions = out
    return n


def build(has_cq, has_ck, has_cv, has_co):
    from contextlib import ExitStack

    nc = bacc.Bacc("TRN2", target_bir_lowering=False, debug=False,
                   num_devices=NCORES)

    x_d = nc.dram_tensor("x", [TL, D], FP32, kind="ExternalInput")
    xr_d = nc.dram_tensor("xr", [TL, D], FP32R, kind="ExternalInput")
    wkv_d = nc.dram_tensor("wkv", [128, 2 * ND * D], BF16, kind="ExternalInput")
    wq_d = nc.dram_tensor("wq", [128, ND * D], BF16, kind="ExternalInput")
    wo_d = nc.dram_tensor("wo", [128, ND * D], BF16, kind="ExternalInput")
    styl_d = nc.dram_tensor("styl", [2 * D], FP32R, kind="ExternalInput")
    cq_d = nc.dram_tensor("cq", [D], FP32R, kind="ExternalInput") if has_cq else None
    ck_d = nc.dram_tensor("ck", [D], FP32R, kind="ExternalInput") if has_ck else None
    cv_d = nc.dram_tensor("cv", [D], FP32R, kind="ExternalInput") if has_cv else None
    co_d = nc.dram_tensor("co", [D], FP32R, kind="ExternalInput") if has_co else None
    out_d = nc.dram_tensor("out", [TL, D], FP32, kind="ExternalOutput")

    def _emit(tc, es):
        constp = es.enter_context(tc.tile_pool(name="const", bufs=1))
        wbig = es.enter_context(tc.tile_pool(name="wbig", bufs=1))
        xio = es.enter_context(tc.tile_pool(name="xio", bufs=2))
        statp = es.enter_context(tc.tile_pool(name="stat", bufs=4))
        dramp = es.enter_context(tc.tile_pool(name="dram", bufs=1, space="DRAM"))
        tp = es.enter_context(tc.tile_pool(name="tp", bufs=2, space="PSUM"))
        pp = es.enter_context(tc.tile_pool(name="pp", bufs=4, space="PSUM"))

        # ------- weight DMAs first, spread across queues -------
        wq = wbig.tile([128, ND * D], BF16)
        nc.gpsimd.dma_start(out=wq[:], in_=wq_d[:])
        wo = wbig.tile([128, ND * D], BF16)
        nc.gpsimd.dma_start(out=wo[:], in_=wo_d[:])

        # ------- constants (sync queue) -------
        identb = constp.tile([128, 128], BF16)
        masks.make_identity(nc, identb[:])
        ident32 = constp.tile([128, 128], FP32)
        masks.make_identity(nc, ident32[:])
        identr = constp.tile([128, 128], FP32R)
        nc.vector.tensor_copy(identr[:], ident32[:])
        ones_row32 = constp.tile([1, 512], FP32)
        nc.vector.memset(ones_row32[:], 1.0)
        ones_row = constp.tile([1, 512], FP32R)
        nc.vector.tensor_copy(ones_row[:], ones_row32[:])
        eps_col = constp.tile([128, 1], FP32)
        nc.vector.memset(eps_col[:], EPS)
        styl_row = constp.tile([1, 2 * D], FP32R)
        nc.sync.dma_start(out=styl_row[:], in_=styl_d[:].unsqueeze(0))
        # broadcast s2/sh2 rows to all partitions via PE
        s2_b = constp.tile([128, D], FP32)
        sh2_b = constp.tile([128, D], FP32)
        for nh in range(4):
            bp = tp.tile([128, 512], FP32, tag="tp")
            nc.tensor.matmul(bp[:], ones_row[:, 0:128],
                             styl_row[:, nh * 512:(nh + 1) * 512])
            dstt = s2_b if nh < 2 else sh2_b
            nc.vector.tensor_copy(dstt[:, (nh % 2) * 512:(nh % 2) * 512 + 512],
                                  bp[:])

        def load_row(pool, dram_ap, n):
            t_ = pool.tile([1, n], FP32R, tag=dram_ap.tensor.name)
            nc.sync.dma_start(out=t_[:], in_=dram_ap.unsqueeze(0))
            return t_

        cq_row = load_row(constp, cq_d.ap(), D) if has_cq else None
        ck_row = load_row(constp, ck_d.ap(), D) if has_ck else None
        cv_row = load_row(constp, cv_d.ap(), D) if has_cv else None
        co_row = load_row(constp, co_d.ap(), D) if has_co else None

        # xnT persistent across stage A + q proj
        es_xnt = ExitStack()
        xntp = es_xnt.enter_context(tc.tile_pool(name="xnT", bufs=1))
        xnt = xntp.tile([128, ND * TL], BF16)

        # wkv lives only through stage A (scalar HWDGE queue, k first)
        es_wkv = ExitStack()
        wkvp = es_wkv.enter_context(tc.tile_pool(name="wkvp", bufs=1))
        wk = wkvp.tile([128, ND * D], BF16)
        nc.scalar.dma_start(out=wk[:], in_=wkv_d[:, 0:ND * D])
        wv = wkvp.tile([128, ND * D], BF16)
        nc.scalar.dma_start(out=wv[:], in_=wkv_d[:, ND * D:])

        # ---------------- stage 0: load x, LN, transpose ----------------
        def em_s0(t):
            xt = xio.tile([128, D], FP32, tag="xin")
            nc.sync.dma_start(out=xt[:], in_=x_d[t * 128:(t + 1) * 128, :])
            st6 = statp.tile([128, 2, 6], FP32, tag="st6")
            nc.vector.bn_stats(st6[:, 0, :], xt[:, 0:512])
            nc.vector.bn_stats(st6[:, 1, :], xt[:, 512:1024])
            agg = statp.tile([128, 2], FP32, tag="agg")
            nc.vector.bn_aggr(agg[:], st6[:])
            rstd = statp.tile([128, 1], FP32, tag="rstd")
            nc.scalar.activation(rstd[:], agg[:, 1:2], AF.Sqrt, bias=eps_col[:])
            nc.vector.reciprocal(rstd[:], rstd[:])
            nmr = statp.tile([128, 1], FP32, tag="nmr")
            nc.vector.scalar_tensor_tensor(nmr[:], agg[:, 0:1], -1.0,
                                           rstd[:], OP.mult, OP.mult)
            xnb = xio.tile([128, D], BF16, tag="xnb")
            nc.scalar.activation(xnb[:], xt[:], AF.Identity,
                                 bias=nmr[:], scale=rstd[:])
            for g in range(2):  # groups of 4 d-chunks
                tpt = tp.tile([128, 512], BF16, tag="tp")
                for i in range(4):
                    dc = g * 4 + i
                    nc.tensor.transpose(tpt[:, i * 128:(i + 1) * 128],
                                        xnb[:, dc * 128:(dc + 1) * 128],
                                        identb[:])
                dst = xnt[:].rearrange("p (dc tt) -> p dc tt", tt=TL)[
                    :, g * 4:(g + 1) * 4, t * 128:(t + 1) * 128]
                src_ = tpt[:].rearrange("p (i c) -> p i c", c=128)
                nc.vector.tensor_copy(dst, src_)

        # ---------------- stage 1: k/v proj + exp + ctx ----------------
        es_kv = ExitStack()
        kvp = es_kv.enter_context(tc.tile_pool(name="kv", bufs=2))
        ctx_sb = constp.tile([128, 8 * 65], FP32)

        def em_kv(t):
            ke = kvp.tile([128, D], BF16, tag="ke")
            va = kvp.tile([128, H * 66], BF16, tag="va")
            lhss = [xnt[:, dc * TL + t * 128: dc * TL + (t + 1) * 128]
                    for dc in range(ND)]
            ps = [pp.tile([128, 512], FP32, tag="pp", name=f"kvps{g}")
                  for g in range(4)]
            base = [0, 512, 0, 512]  # col offsets within wk / wv
            wt = [wk, wk, wv, wv]
            crow = [ck_row, ck_row, cv_row, cv_row]
            cflag = [has_ck, has_ck, has_cv, has_cv]
            for g in range(4):  # k first: its DMA lands before v
                for dc in range(ND):
                    nc.tensor.matmul(
                        ps[g][:], lhss[dc],
                        wt[g][:, base[g] + dc * D:base[g] + dc * D + 512],
                        start=(dc == 0), stop=(dc == ND - 1 and not cflag[g]))
                if cflag[g]:
                    jh = g % 2
                    nc.tensor.matmul(ps[g][:], ones_row[:, 0:128],
                                     crow[g][:, jh * 512:(jh + 1) * 512],
                                     start=False, stop=True)
            # epilogues: exp(k) on scalar, v copy on vector
            for jh in range(2):
                nc.scalar.activation(ke[:, jh * 512:(jh + 1) * 512],
                                     ps[jh][:], AF.Exp)
            for jh in range(2):
                nc.vector.tensor_copy(
                    va[:].rearrange("p (h l) -> p h l", l=66)[
                        :, jh * 8:(jh + 1) * 8, 0:64],
                    ps[2 + jh][:].rearrange("p (h l) -> p h l", l=64))
            if t < 2:  # pool has 2 bufs; ones col survives v copies
                nc.vector.memset(
                    va[:].rearrange("p (h l) -> p h l", l=66)[:, :, 64:65], 1.0)
            # ctx: one single-shot matmul per psum tile; heads pack pairwise
            for hp in range(H // 2):
                cp = tp.tile([128, 512], FP32, tag="tp")
                for par in range(2):
                    h = 2 * hp + par
                    nc.tensor.matmul(cp[par * 64:par * 64 + 64, 0:65],
                                     ke[:, h * 64:(h + 1) * 64],
                                     va[:, h * 66:h * 66 + 65])
                if t == 0:
                    nc.vector.tensor_copy(
                        ctx_sb[:, hp * 65:(hp + 1) * 65], cp[:, 0:65])
                else:
                    nc.vector.tensor_tensor(
                        ctx_sb[:, hp * 65:(hp + 1) * 65],
                        ctx_sb[:, hp * 65:(hp + 1) * 65],
                        cp[:, 0:65], OP.add)

        for tt in range(NT + 2):
            if tt < NT:
                em_s0(tt)
            if tt >= 2:
                em_kv(tt - 2)

        # ---------------- stage ctx + pairwise AllReduce ----------------
        cc_in = dramp.tile([H, 64, 65], FP32)
        cc_out = dramp.tile([H, 64, 65], FP32)
        for q in range(2):
            nc.sync.dma_start(
                out=cc_in[:].rearrange("(g q) d l -> q d g l", q=2)[q],
                in_=ctx_sb[q * 64:(q + 1) * 64, :].rearrange(
                    "d (g l) -> d g l", l=65))
        if USE_COLLECTIVE:
            nc.gpsimd.collective_compute(
                "AllReduce", OP.add,
                replica_groups=[[0, 1], [2, 3], [4, 5], [6, 7]],
                ins=[cc_in.opt()], outs=[cc_out.opt()])
        else:
            nc.sync.dma_start(out=cc_out[:], in_=cc_in[:])
        es_kv.close()
        es_wkv.close()

        # ---------------- stage 2: q proj + exp (overlaps collective) ----------------
        es_qt = ExitStack()
        qtp = es_qt.enter_context(tc.tile_pool(name="qT", bufs=1, side="right"))
        qT = qtp.tile([128, ND * TL], BF16)  # j-chunk jc at cols jc*TL

        def q_th(th):
            for jc in range(ND):
                qps = pp.tile([128, 512], FP32, tag="pp")
                for dc in range(ND):
                    nc.tensor.matmul(
                        qps[:],
                        wq[:, dc * D + jc * 128:dc * D + jc * 128 + 128],
                        xnt[:, dc * TL + th * 512:dc * TL + (th + 1) * 512],
                        start=(dc == 0), stop=(dc == ND - 1 and not has_cq))
                if has_cq:
                    nc.tensor.matmul(qps[:],
                                     cq_row[:, jc * 128:(jc + 1) * 128],
                                     ones_row[:], start=False, stop=True)
                nc.scalar.activation(
                    qT[:, jc * TL + th * 512:jc * TL + (th + 1) * 512],
                    qps[:], AF.Exp)

        q_th(0)  # overlaps the collective

        # ---------------- readback + normalize ctx ----------------
        ctxn = constp.tile([128, 8 * 65], FP32)
        for q in range(2):
            nc.sync.dma_start(
                out=ctxn[q * 64:(q + 1) * 64, :].rearrange(
                    "d (g l) -> d g l", l=65),
                in_=cc_out[:].rearrange("(g q) d l -> q d g l", q=2)[q])
        rk = statp.tile([128, 8], FP32, tag="rk")
        nc.vector.reciprocal(
            rk[:], ctxn[:].rearrange("p (g l) -> p g l", l=65)[:, :, 64])
        ctx_aug = constp.tile([128, 8 * 66], BF16)
        for g in range(8):
            nc.vector.tensor_scalar(ctx_aug[:, g * 66:g * 66 + 64],
                                    ctxn[:, g * 65:g * 65 + 64],
                                    rk[:, g:g + 1], None, OP.mult)
        nc.vector.memset(
            ctx_aug[:].rearrange("p (g l) -> p g l", l=66)[:, :, 64:65], 1.0)

        # ---------------- stage 5+6: y / LN+styl+silu / out ----------------
        es_out = ExitStack()
        hyp = es_out.enter_context(tc.tile_pool(name="hy", bufs=3))
        op_ = es_out.enter_context(tc.tile_pool(name="op", bufs=2, space="PSUM"))

        yts, hsbs, mvs, rstds = {}, {}, {}, {}

        def em_y(t):
            # prefetch x for the residual (consumed in em_out)
            xt2 = xio.tile([128, D], FP32R, tag="xres", bufs=6)
            nc.gpsimd.dma_start(out=xt2[:], in_=xr_d[t * 128:(t + 1) * 128, :])
            yt = hyp.tile([128, D], FP32, tag="yt", bufs=5)
            rq = statp.tile([128, 16], FP32, tag="rq")
            yts[t] = (yt, xt2)
            if Y_GROUPED:
                for g in range(4):
                    par = g % 2
                    jc0 = 4 * (g // 2)
                    yp = pp.tile([128, 512], FP32, tag="pp")
                    for i in range(4):
                        jc = jc0 + i
                        nc.tensor.matmul(
                            yp[:, i * 65:i * 65 + 65],
                            qT[par * 64:par * 64 + 64,
                               jc * TL + t * 128:jc * TL + (t + 1) * 128],
                            ctx_aug[par * 64:par * 64 + 64,
                                    jc * 66:jc * 66 + 65],
                            start=(i == 0), stop=(i == 3),
                            skip_group_check=True)
                    rq_sl = rq[:].rearrange("p (a two) -> p a two", two=2)[
                        :, jc0:jc0 + 4, par]
                    nc.vector.reciprocal(
                        rq_sl,
                        yp[:, 0:260].rearrange("p (i c) -> p i c", c=65)[:, :, 64])
                    ydst = yt[:].rearrange("p (a two c) -> p a two c",
                                           two=2, c=64)[:, jc0:jc0 + 4, par, :]
                    nc.vector.tensor_tensor(
                        ydst,
                        yp[:, 0:260].rearrange("p (i c) -> p i c", c=65)[
                            :, :, 0:64],
                        rq_sl.unsqueeze(2).broadcast_to([128, 4, 64]),
                        OP.mult)
            else:
                for h in range(H):
                    par = h % 2
                    yp = pp.tile([128, 512], FP32, tag="pp")
                    nc.tensor.matmul(
                        yp[:, 0:65],
                        qT[par * 64:par * 64 + 64,
                           (h // 2) * TL + t * 128:(h // 2) * TL + (t + 1) * 128],
                        ctx_aug[par * 64:par * 64 + 64,
                                (h // 2) * 66:(h // 2) * 66 + 65])
                    nc.vector.reciprocal(rq[:, h:h + 1], yp[:, 64:65])
                    if h % 2 == 0:
                        nc.scalar.activation(yt[:, h * 64:(h + 1) * 64],
                                             yp[:, 0:64], AF.Identity,
                                             scale=rq[:, h:h + 1])
                    else:
                        nc.vector.tensor_scalar(yt[:, h * 64:(h + 1) * 64],
                                                yp[:, 0:64], rq[:, h:h + 1],
                                                None, OP.mult)

        def em_bn(t):
            yt, _ = yts[t]
            blk, i = t // 4, t % 4
            if i == 0:
                mvs[blk] = statp.tile([128, 8], FP32, tag="mv8", name="mv8")
            st6 = statp.tile([128, 2, 6], FP32, tag="st6")
            nc.vector.bn_stats(st6[:, 0, :], yt[:, 0:512])
            nc.vector.bn_stats(st6[:, 1, :], yt[:, 512:1024])
            nc.vector.bn_aggr(mvs[blk][:, 2 * i:2 * i + 2], st6[:])

        def em_sqrt(blk):
            mv8 = mvs[blk]
            srt = statp.tile([128, 4], FP32, tag="srt")
            nc.scalar.activation(
                srt[:],
                mv8[:].rearrange("p (a two) -> p a two", two=2)[:, :, 1],
                AF.Sqrt, bias=eps_col[:])
            rstd4 = statp.tile([128, 4], FP32, tag="rstd4")
            nc.vector.reciprocal(rstd4[:], srt[:])
            rstds[blk] = rstd4

        def em_ln(t):
            yt, _ = yts[t]
            blk, i = t // 4, t % 4
            mean = mvs[blk][:, 2 * i:2 * i + 1]
            rstd = rstds[blk][:, i:i + 1]
            ln = hyp.tile([128, D], FP32, tag="ln", bufs=2)
            nc.vector.tensor_scalar(ln[:], yt[:], mean, rstd,
                                    OP.subtract, OP.mult)
            nc.gpsimd.tensor_tensor(ln[:], ln[:], s2_b[:], OP.mult)
            nc.vector.tensor_tensor(ln[:], ln[:], sh2_b[:], OP.add)
            hsb = hyp.tile([128, D], BF16, tag="hsb")
            nc.scalar.activation(hsb[:], ln[:], AF.Silu)
            hsbs[t] = hsb

        def em_out(t):
            hsb = hsbs.pop(t)
            _, xt2 = yts.pop(t)
            hst = hyp.tile([128, D], BF16, tag="hst", bufs=2)
            for g in range(2):
                tpt = tp.tile([128, 512], BF16, tag="tp", name="tptb")
                for i in range(4):
                    dc = g * 4 + i
                    nc.tensor.transpose(tpt[:, i * 128:(i + 1) * 128],
                                        hsb[:, dc * 128:(dc + 1) * 128],
                                        identb[:])
                nc.scalar.copy(hst[:, g * 512:(g + 1) * 512], tpt[:])
            ops = [op_.tile([128, 512], FP32, tag="op", name=f"outps{j}")
                   for j in range(2)]
            for dc in range(ND):
                for jh in range(2):
                    nc.tensor.matmul(
                        ops[jh][:], hst[:, dc * 128:(dc + 1) * 128],
                        wo[:, dc * D + jh * 512:dc * D + (jh + 1) * 512],
                        start=(dc == 0), stop=False)
            for jh in range(2):  # residual: += ident.T @ x  (fp32r, full rate)
                nc.tensor.matmul(ops[jh][:], identr[:],
                                 xt2[:, jh * 512:(jh + 1) * 512],
                                 start=False, stop=(not has_co))
            if has_co:
                for jh in range(2):
                    nc.tensor.matmul(ops[jh][:], ones_row[:, 0:128],
                                     co_row[:, jh * 512:(jh + 1) * 512],
                                     start=False, stop=True)
            fin = xio.tile([128, D], FP32, tag="fin", bufs=2)
            for jh in range(2):
                nc.scalar.copy(fin[:, jh * 512:(jh + 1) * 512], ops[jh][:])
            nc.gpsimd.dma_start(out=out_d[t * 128:(t + 1) * 128, :], in_=fin[:])

        for tt in range(NT + 5):
            if tt < NT:
                if tt % 4 == 0 and tt // 4 + 1 < 4:
                    q_th(tt // 4 + 1)
                em_y(tt)
                em_bn(tt)
                if tt % 4 == 3:
                    em_sqrt(tt // 4)
            if 4 <= tt < NT + 4:
                em_ln(tt - 4)
            if tt >= 5:
                em_out(tt - 5)
        es_out.close()
        es_xnt.close()
        es_qt.close()

    with tile.TileContext(nc) as tc, ExitStack() as es:
        _emit(tc, es)
    nc.compile()
    _legalize_waits(nc)
    return nc


def _silu(z):
    return z / (1.0 + np.exp(-z))


def kernel(**inputs):
    x = np.asarray(inputs["x"], np.float32)
    emb = np.asarray(inputs["emb"], np.float32)
    gate_msa = np.asarray(inputs["gate_msa"], np.float32)
    norm_g = np.asarray(inputs["norm_g"], np.float32)
    norm_b = np.asarray(inputs["norm_b"], np.float32)
    Wq = np.asarray(inputs["Wq"], np.float32)
    bq = np.asarray(inputs["bq"], np.float32)
    Wk = np.asarray(inputs["Wk"], np.float32)
    bk = np.asarray(inputs["bk"], np.float32)
    Wv = np.asarray(inputs["Wv"], np.float32)
    bv = np.asarray(inputs["bv"], np.float32)
    emb_W = np.asarray(inputs["emb_W"], np.float32)
    emb_b = np.asarray(inputs["emb_b"], np.float32)
    sn_g = np.asarray(inputs["sn_g"], np.float32)
    sn_b = np.asarray(inputs["sn_b"], np.float32)
    out_W = np.asarray(inputs["out_W"], np.float32)
    out_b = np.asarray(inputs["out_b"], np.float32)

    import ml_dtypes

    def _pack(w):  # [D, N] -> [128, ND * N] with d-chunk dc at cols dc*N
        n = w.shape[1]
        return np.ascontiguousarray(
            w.reshape(ND, 128, n).transpose(1, 0, 2).reshape(128, ND * n))

    # fold layernorm affine into projection weights
    wq_f = _pack((norm_g[:, None] * Wq).astype(ml_dtypes.bfloat16))
    wk_f = norm_g[:, None] * Wk
    wv_f = norm_g[:, None] * Wv
    wkv_f = np.ascontiguousarray(np.concatenate(
        [_pack(wk_f.astype(ml_dtypes.bfloat16)),
         _pack(wv_f.astype(ml_dtypes.bfloat16))], axis=1))
    cq = norm_b @ Wq + bq
    ck = norm_b @ Wk + bk
    cv = norm_b @ Wv + bv

    # emb MLP + stylization constants on host (tiny)
    emb_out = _silu(emb[:, 0, :]) @ emb_W + emb_b       # [B, 2D]
    scale, shift = emb_out[:, :D], emb_out[:, D:]
    s2 = sn_g[None, :] * (1.0 + scale)                  # [B, D]
    sh2 = sn_b[None, :] * (1.0 + scale) + shift

    gate = gate_msa[:, 0, :]                            # [B, D]
    co = out_b[None, :] * gate                          # [B, D]

    flags = (bool(np.any(cq)), bool(np.any(ck)), bool(np.any(cv)),
             bool(np.any(co)))
    if flags not in _CACHE:
        _CACHE[flags] = build(*flags)
    nc = _CACHE[flags]

    in_maps = []
    for c in range(NCORES):
        b, half = c // 2, c % 2
        wo_g = _pack((out_W * gate[b][None, :]).astype(ml_dtypes.bfloat16))
        styl = np.concatenate([s2[b], sh2[b]]).astype(np.float32)
        m = {
            "x": np.ascontiguousarray(x[b, half * TL:(half + 1) * TL, :]),
            "xr": np.ascontiguousarray(x[b, half * TL:(half + 1) * TL, :]),
            "wq": wq_f, "wkv": wkv_f, "wo": wo_g,
            "styl": np.ascontiguousarray(styl),
        }
        if flags[0]:
            m["cq"] = cq
        if flags[1]:
            m["ck"] = ck
        if flags[2]:
            m["cv"] = cv
        if flags[3]:
            m["co"] = np.ascontiguousarray(co[b])
        in_maps.append(m)

    res = run_bass_kernel_spmd(nc, in_maps, core_ids=list(range(NCORES)),
                               **_RUN_KW)
    kernel.last_result = res
    out = np.stack([res.results[c]["out"] for c in range(NCORES)])
    return out.reshape(B, 2, TL, D).reshape(B, T, D)


_RUN_KW = {}
kernel.last_result = None
